# revision 18
# baseline (speedup 1.0000x reference)
"""GATv2 4-layer GNN on 8 TRN2 NeuronCores.

Sharding: nodes LPT-packed into 240 balanced (core,window) bins of 128 nodes;
edges (incl. self loops) assigned to the bin of their dst. Per layer: local
node matmuls -> AllGather of bf16 xl -> per-window edge phase:
  - one batched dma_gather of xl[src] rows (A2, [e,c] layout)
  - host-precomputed onehot matrices (both orientations) streamed from DRAM
  - z built TRANSPOSED [c,e] in PSUM: xr-expand matmul + A2-transpose matmuls
  - leaky_relu on ScalarE -> logits via N=4 PE matmuls with att folded into
    a block-diagonal rhs -> exp on ScalarE writes p into pa columns
  - message weighting: 4x 4x-mode tensor_scalar per chunk; scatter via
    onehot matmul into PSUM acc; BN stats via ones-column matmuls.
BN-stat AllReduce; fused BN+elu; graph mean-pool + 2-layer MLP replicated.
"""
import heapq

import numpy as np
import ml_dtypes

import concourse.bass as bass
import concourse.bacc as bacc
import concourse.mybir as mybir
import concourse.tile as tile
from concourse.bass_utils import run_bass_kernel_spmd
from concourse.masks import make_identity

P = 128
NCORE = 8
N = 30000
E = 300000
G = 128
IN = 128
HID = 64
HEADS = 4
NCLS = 10
EPS = 1e-5
SLOPE = 0.2
NPC = N // NCORE            # 3750 real nodes per core
W = 30                      # windows per core
NPAD = W * P                # 3840 padded nodes per core
ROWS = NCORE * NPAD         # 30720 rows in the allgathered xl
OC = HEADS * HID            # 256
# (layer, input-channel-chunk) order for weight / hT chunk layouts
IC_CHUNKS = [(0, 0), (1, 0), (1, 1), (2, 0), (2, 1), (3, 0), (3, 1)]

f32 = mybir.dt.float32
bf16 = mybir.dt.bfloat16
i32 = mybir.dt.int32
i16 = mybir.dt.int16
bf = ml_dtypes.bfloat16


def _pack_nodes(deg):
    """LPT-pack nodes into NCORE*W bins of <=128 nodes, balancing edge load."""
    nbins = NCORE * W
    order = np.argsort(-deg, kind="stable")
    heap = [(0, b) for b in range(nbins)]
    heapq.heapify(heap)
    counts = np.zeros(nbins, np.int64)
    load = np.zeros(nbins, np.int64)
    node_bin = np.empty(N, np.int64)
    node_pos = np.empty(N, np.int64)
    for n in order:
        while True:
            l, b = heapq.heappop(heap)
            if counts[b] < P:
                break
        node_bin[n] = b
        node_pos[n] = counts[b]
        counts[b] += 1
        load[b] += deg[n]
        if counts[b] < P:
            heapq.heappush(heap, (load[b], b))
    return node_bin, node_pos


def _host_prep(inputs):
    x = np.asarray(inputs["x"], np.float32)
    ei = np.asarray(inputs["edge_index"], np.int64)
    batch = np.asarray(inputs["batch"], np.int64)

    src = ei[0]
    dst = ei[1]
    deg = np.bincount(dst, minlength=N) + 1  # +1 self loop
    node_bin, node_pos = _pack_nodes(deg)
    row_of = (node_bin // W) * NPAD + (node_bin % W) * P + node_pos

    # unified edge list: self loops first, then real edges; bin of dst
    e_bin = np.concatenate([node_bin, node_bin[dst]])
    e_srcrow = np.concatenate([row_of, row_of[src]]).astype(np.int64)
    e_dstpos = np.concatenate([node_pos, node_pos[dst]]).astype(np.int64)
    order = np.argsort(e_bin, kind="stable")
    cnt_b = np.bincount(e_bin, minlength=NCORE * W)
    cpw = int(-(-int(cnt_b.max()) // P))     # chunks per window
    NIDX = cpw * P                           # gather idxs per window

    idx_tab = np.zeros((NCORE, P, W * (NIDX // 16)), np.int16)
    ohde_tab = np.zeros((NCORE, P, W * cpw * P), bf)
    ohed_tab = np.zeros((NCORE, P, W * cpw * P), bf)

    bin_starts = np.zeros(NCORE * W + 1, np.int64)
    np.cumsum(cnt_b, out=bin_starts[1:])
    se_srcrow = e_srcrow[order]
    se_dstpos = e_dstpos[order]
    for b in range(NCORE * W):
        c, w = b // W, b % W
        lo, hi = bin_starts[b], bin_starts[b + 1]
        k = int(hi - lo)
        srk = se_srcrow[lo:hi]
        dpk = se_dstpos[lo:hi]
        lin = np.arange(k)
        jj = lin // P
        pp = lin % P
        col = w * (NIDX // 16) + lin // 16
        prow = lin % 16
        for m in range(8):
            idx_tab[c, prow + 16 * m, col] = srk.astype(np.int16)
        ohde_tab[c, dpk, w * cpw * P + lin] = 1.0
        ohed_tab[c, pp, (w * cpw + jj) * P + dpk] = 1.0

    # graph indicator per core: [P pos, W*P] (win w block cols -> graph onehot)
    cores_n = node_bin // W
    wins_n = node_bin % W
    ind_tab = np.zeros((NCORE, P, W * P), bf)
    ind_tab[cores_n, node_pos, wins_n * P + batch] = 1.0

    # x transposed per core, permuted order: [IN, NPAD]
    xT = np.zeros((NCORE, IN, NPAD), bf)
    cols = wins_n * P + node_pos
    xT[cores_n, :, cols] = x.astype(bf)

    # weights (identical on all cores)
    wl_tab = np.zeros((P, len(IC_CHUNKS) * OC), bf)
    wr_tab = np.zeros((P, len(IC_CHUNKS) * OC), bf)
    for j, (l, cc) in enumerate(IC_CHUNKS):
        wl = np.asarray(inputs[f"Wl{l}"], np.float32)
        wr = np.asarray(inputs[f"Wr{l}"], np.float32)
        wl_tab[:, j * OC:(j + 1) * OC] = wl[cc * P:(cc + 1) * P, :].astype(bf)
        wr_tab[:, j * OC:(j + 1) * OC] = wr[cc * P:(cc + 1) * P, :].astype(bf)
    # block-diagonal att rhs: [c_part, 4 layers * 2 cc * 4 heads]
    attbd_tab = np.zeros((P, 4 * 2 * HEADS), bf)
    for l in range(4):
        a = np.asarray(inputs[f"att{l}"], np.float32)  # [HEADS, HID]
        for cc in range(2):
            for cp in range(P):
                ch = cc * P + cp
                h = ch // HID
                attbd_tab[cp, l * 8 + cc * 4 + h] = a[h, ch % HID]
    g_tab = np.ones((P, 7), np.float32)
    b_tab = np.zeros((P, 7), np.float32)
    OCC = [(0, 0), (0, 1), (1, 0), (1, 1), (2, 0), (2, 1), (3, 0)]
    for j, (l, cc) in enumerate(OCC):
        g = np.asarray(inputs[f"bng{l}"], np.float32)
        bb = np.asarray(inputs[f"bnb{l}"], np.float32)
        seg = g[cc * P:(cc + 1) * P]
        g_tab[: len(seg), j] = seg
        segb = bb[cc * P:(cc + 1) * P]
        b_tab[: len(segb), j] = segb

    fc1w = np.asarray(inputs["fc1_w"], np.float32).astype(bf)
    fc2w = np.asarray(inputs["fc2_w"], np.float32).astype(bf)
    fc1b_rep = np.tile(np.asarray(inputs["fc1_b"], np.float32).reshape(1, HID), (P, 1))
    fc2b_rep = np.tile(np.asarray(inputs["fc2_b"], np.float32).reshape(1, NCLS), (P, 1))
    gcnt = np.bincount(batch, minlength=G).astype(np.float32)
    cnt_recip = (1.0 / np.maximum(gcnt, 1.0)).reshape(P, 1).astype(np.float32)

    in_maps = []
    for c in range(NCORE):
        in_maps.append(dict(
            xT=np.ascontiguousarray(xT[c]),
            idx_tab=np.ascontiguousarray(idx_tab[c]),
            ohde_tab=np.ascontiguousarray(ohde_tab[c]),
            ohed_tab=np.ascontiguousarray(ohed_tab[c]),
            ind_tab=np.ascontiguousarray(ind_tab[c]),
            wl_tab=wl_tab, wr_tab=wr_tab, attbd_tab=attbd_tab,
            g_tab=g_tab, b_tab=b_tab,
            fc1w=fc1w, fc2w=fc2w, fc1b_rep=fc1b_rep, fc2b_rep=fc2b_rep,
            cnt_recip=cnt_recip,
        ))
    return in_maps, cpw


def _build_program(cpw):
    NIDX = cpw * P
    nc = bacc.Bacc(num_devices=NCORE, num_swdge_queues=4,
                   dynamic_dma_scratch_size=32768)
    ap = mybir.AluOpType
    act = mybir.ActivationFunctionType
    rg = [list(range(NCORE))]

    # --- external inputs ---
    xT_in = nc.dram_tensor("xT", [IN, NPAD], bf16, kind="ExternalInput")
    idx_in = nc.dram_tensor("idx_tab", [P, W * (NIDX // 16)], i16, kind="ExternalInput")
    ohde_in = nc.dram_tensor("ohde_tab", [P, W * cpw * P], bf16, kind="ExternalInput")
    ohed_in = nc.dram_tensor("ohed_tab", [P, W * cpw * P], bf16, kind="ExternalInput")
    ind_in = nc.dram_tensor("ind_tab", [P, W * P], bf16, kind="ExternalInput")
    wl_in = nc.dram_tensor("wl_tab", [P, len(IC_CHUNKS) * OC], bf16, kind="ExternalInput")
    wr_in = nc.dram_tensor("wr_tab", [P, len(IC_CHUNKS) * OC], bf16, kind="ExternalInput")
    attbd_in = nc.dram_tensor("attbd_tab", [P, 32], bf16, kind="ExternalInput")
    g_in = nc.dram_tensor("g_tab", [P, 7], f32, kind="ExternalInput")
    b_in = nc.dram_tensor("b_tab", [P, 7], f32, kind="ExternalInput")
    fc1w_in = nc.dram_tensor("fc1w", [HID, HID], bf16, kind="ExternalInput")
    fc2w_in = nc.dram_tensor("fc2w", [HID, NCLS], bf16, kind="ExternalInput")
    fc1b_in = nc.dram_tensor("fc1b_rep", [P, HID], f32, kind="ExternalInput")
    fc2b_in = nc.dram_tensor("fc2b_rep", [P, NCLS], f32, kind="ExternalInput")
    cnt_in = nc.dram_tensor("cnt_recip", [P, 1], f32, kind="ExternalInput")
    out_dram = nc.dram_tensor("out", [G, NCLS], f32, kind="ExternalOutput")

    with tile.TileContext(nc) as tc:
        with (
            tc.tile_pool(name="persist", bufs=1) as pers,
            tc.tile_pool(name="sbuf", bufs=3) as sb,
            tc.tile_pool(name="stage", bufs=2) as stg,
            tc.tile_pool(name="psz", bufs=2, space="PSUM") as psz_pool,
            tc.tile_pool(name="psacc", bufs=2, space="PSUM") as psacc,
            tc.tile_pool(name="pssm", bufs=1, space="PSUM") as pssm,
            tc.tile_pool(name="psstat", bufs=1, space="PSUM") as psstat,
            tc.tile_pool(name="dram", bufs=1, space="DRAM") as dr,
        ):
            # ---- persistent tiles / table loads ----
            ident_f = pers.tile([P, P], f32)
            make_identity(nc, ident_f[:])
            ident_bf = pers.tile([P, P], bf16)
            nc.vector.tensor_copy(ident_bf[:], ident_f[:])
            alpha_col = pers.tile([P, 1], f32)
            nc.vector.memset(alpha_col[:], SLOPE)
            ones_f = pers.tile([1, P], f32)
            nc.vector.memset(ones_f[:], 1.0)
            ones_col = pers.tile([P, 1], bf16)
            nc.vector.memset(ones_col[:], 1.0)
            zeros4 = pers.tile([P, 4], bf16)
            nc.vector.memset(zeros4[:], 0.0)

            def load(name, handle, shape, dt):
                t = pers.tile(shape, dt, name=name)
                nc.sync.dma_start(t[:], handle[:, :])
                return t
            xT_sb = load("xT_sb", xT_in, [IN, NPAD], bf16)
            idx_sb = load("idx_sb", idx_in, [P, W * (NIDX // 16)], i16)
            ind_sb = load("ind_sb", ind_in, [P, W * P], bf16)
            wl_sb = load("wl_sb", wl_in, [P, len(IC_CHUNKS) * OC], bf16)
            wr_sb = load("wr_sb", wr_in, [P, len(IC_CHUNKS) * OC], bf16)
            attbd_sb = load("attbd_sb", attbd_in, [P, 32], bf16)
            g_sb = load("g_sb", g_in, [P, 7], f32)
            b_sb = load("b_sb", b_in, [P, 7], f32)
            fc1w_sb = load("fc1w_sb", fc1w_in, [HID, HID], bf16)
            fc2w_sb = load("fc2w_sb", fc2w_in, [HID, NCLS], bf16)
            fc1b_sb = load("fc1b_sb", fc1b_in, [P, HID], f32)
            fc2b_sb = load("fc2b_sb", fc2b_in, [P, NCLS], f32)
            cnt_sb = load("cnt_sb", cnt_in, [P, 1], f32)

            hT_a = pers.tile([P, NPAD], bf16)      # h^T chunk c0:128
            hT_b = pers.tile([P, NPAD], bf16)      # h^T chunk c128:256
            hpre_a = pers.tile([P, NPAD], bf16)    # pre-BN h^T chunk 0
            hpre_b = pers.tile([P, NPAD], bf16)
            h3_pre = pers.tile([P, W * HID], f32)  # layer3 pre-BN, [n, c] layout
            h3_bn = pers.tile([P, W * HID], bf16)
            xr_all = pers.tile([P, W * OC], bf16)
            xl_all = pers.tile([P, W * OC], bf16)

            # DRAM scratch
            xl_loc = dr.tile([NPAD, OC], bf16)
            xl_fulls = [dr.tile([ROWS, OC], bf16, addr_space="Shared", name=f"xl_full{i}")
                        for i in range(4)]
            stats_i = dr.tile([P, 4], f32)
            stats_os = [dr.tile([P, 4], f32, addr_space="Shared", name=f"stats_o{i}")
                        for i in range(4)]
            pool_i = dr.tile([P, HID], f32)
            pool_o = dr.tile([P, HID], f32, addr_space="Shared")

            # gather segments of <=8 chunks (1024-idx dma_gather limit),
            # each split into compute groups of <=4 chunks
            segs = []
            jb = 0
            while jb < cpw:
                sl = min(8, cpw - jb)
                segs.append((jb, sl))
                jb += sl
            groups = []   # (jbase_global, gs, seg_index, jbase_in_seg)
            for si, (jb, sl) in enumerate(segs):
                q = 0
                while q < sl:
                    gs = min(4, sl - q)
                    groups.append((jb + q, gs, si, q))
                    q += gs

            for l in range(4):
                ic_chunks = [j for j, (ll, _) in enumerate(IC_CHUNKS) if ll == l]
                hts = [xT_sb] if l == 0 else [hT_a, hT_b]

                # ---- node phase: xl = h @ Wl, xr = h @ Wr ----
                for w in range(W):
                    ps_xl = pssm.tile([P, OC], f32, name="ps_xl", tag="psA")
                    ps_xr = psacc.tile([P, OC], f32, name="ps_xr", tag="acc")
                    for i, j in enumerate(ic_chunks):
                        lhs = hts[i][:, w * P:(w + 1) * P]
                        nc.tensor.matmul(ps_xl[:], lhsT=lhs, rhs=wl_sb[:, j * OC:(j + 1) * OC],
                                         start=(i == 0), stop=(i == len(ic_chunks) - 1))
                        nc.tensor.matmul(ps_xr[:], lhsT=lhs, rhs=wr_sb[:, j * OC:(j + 1) * OC],
                                         start=(i == 0), stop=(i == len(ic_chunks) - 1))
                    xlsl = xl_all[:, w * OC:(w + 1) * OC]
                    nc.scalar.copy(xlsl, ps_xl[:])
                    nc.sync.dma_start(xl_loc[w * P:(w + 1) * P, :], xlsl)
                    nc.vector.tensor_copy(xr_all[:, w * OC:(w + 1) * OC], ps_xr[:])

                # ---- allgather xl ----
                xl_full = xl_fulls[l]
                nc.gpsimd.collective_compute(
                    "AllGather", ap.bypass, replica_groups=rg,
                    ins=[xl_loc[:]], outs=[xl_full[:]],
                )

                # ---- edge phase ----
                attl = attbd_sb[:, l * 8:l * 8 + 8]   # [c, cc*4+h]
                stat_ps = psstat.tile([P, 4], f32, name="stat_ps", tag="stat")
                # open + zero the stat psum bank once; windows accumulate into
                # it with start=False; the last stat matmul closes the group
                nc.tensor.matmul(stat_ps[:], lhsT=ident_bf[:], rhs=zeros4[:],
                                 start=True, stop=False)
                for w in range(W):
                    # batched gathers of xl[src] (<=1024 idxs each)
                    a2tiles = []
                    for si, (jb, sl) in enumerate(segs):
                        A2s = stg.tile([P, sl * OC], bf16, name=f"A2s{si}",
                                       tag=f"A2{si}")
                        nidx_s = sl * P
                        c0 = (w * cpw + jb) * 8
                        nc.gpsimd.dma_gather(
                            A2s[:].rearrange("p (j c) -> p j c", c=OC),
                            xl_full[:, :],
                            idx_sb[:, c0:c0 + nidx_s // 16],
                            nidx_s, nidx_s, OC,
                            queue_num=0,
                        )
                        a2tiles.append(A2s)
                    ohde_w = stg.tile([P, cpw * P], bf16, name="ohde_w", tag="ohde")
                    nc.sync.dma_start(ohde_w[:], ohde_in[:, w * cpw * P:(w + 1) * cpw * P])
                    ohed_w = stg.tile([P, cpw * P], bf16, name="ohed_w", tag="ohed")
                    nc.sync.dma_start(ohed_w[:], ohed_in[:, w * cpw * P:(w + 1) * cpw * P])

                    acc = psacc.tile([P, OC + 4], f32, name="acc", tag="acc")
                    xrw0 = xr_all[:, w * OC:w * OC + P]
                    xrw1 = xr_all[:, w * OC + P:(w + 1) * OC]
                    for (j0, gs, si, qb) in groups:
                        A2t = a2tiles[si]
                        E_g = gs * P
                        zt = psz_pool.tile([P, 2 * E_g], f32, name="zt", tag="zt")
                        lg = pssm.tile([P, 4 * gs], f32, name="lg", tag="psA")
                        # z[c,e] = xr[dst_e,c] + xl[src_e,c], transposed, built
                        # per chunk region: xr-expand opens the psum group,
                        # the A2 transpose-matmul closes it (one open group
                        # per psum bank at a time).
                        for q in range(gs):
                            j = j0 + q
                            jl = qb + q
                            ohq = ohde_w[:, j * P:(j + 1) * P]
                            for cc, xrw in ((0, xrw0), (1, xrw1)):
                                reg = slice(cc * E_g + q * P, cc * E_g + (q + 1) * P)
                                nc.tensor.matmul(zt[:, reg], lhsT=xrw, rhs=ohq,
                                                 start=True, stop=False)
                                nc.tensor.matmul(
                                    zt[:, reg],
                                    lhsT=A2t[:, jl * OC + cc * P:jl * OC + (cc + 1) * P],
                                    rhs=ident_bf[:],
                                    start=False, stop=True)
                        # leaky relu -> bf16
                        zl = sb.tile([P, 2 * E_g], bf16, name="zl", bufs=2)
                        nc.scalar.activation(zl[:, 0:E_g], zt[:, 0:E_g], act.Prelu,
                                             bias=0.0, scale=1.0, alpha=alpha_col[:, :1])
                        nc.scalar.activation(zl[:, E_g:2 * E_g], zt[:, E_g:2 * E_g],
                                             act.Prelu,
                                             bias=0.0, scale=1.0, alpha=alpha_col[:, :1])
                        # logits: lg[e, 4] = sum_c zl[c,e] * att_bd[c,h]
                        for q in range(gs):
                            nc.tensor.matmul(lg[:, 4 * q:4 * q + 4],
                                             lhsT=zl[:, q * P:(q + 1) * P],
                                             rhs=attl[:, 0:4], start=True, stop=False)
                            nc.tensor.matmul(lg[:, 4 * q:4 * q + 4],
                                             lhsT=zl[:, E_g + q * P:E_g + (q + 1) * P],
                                             rhs=attl[:, 4:8], start=False, stop=True)
                        # p = exp(lg) (f32 for tensor_scalar), bf16 copy into pa
                        p_f = sb.tile([P, 4 * gs], f32, name="p_f", bufs=2)
                        nc.scalar.activation(p_f[:], lg[:], act.Exp)
                        pa = sb.tile([P, gs * (OC + 8)], bf16, name="pa", bufs=2)
                        pav = pa[:].rearrange("p (q d) -> p q d", q=gs)
                        nc.vector.tensor_copy(
                            pav[:, :, OC:OC + 4],
                            p_f[:].rearrange("p (q h) -> p q h", q=gs))
                        # weighted messages
                        for q in range(gs):
                            jl = qb + q
                            for h in range(HEADS):
                                nc.vector.tensor_scalar(
                                    out=pa[:, q * (OC + 8) + h * HID:
                                           q * (OC + 8) + (h + 1) * HID],
                                    in0=A2t[:, jl * OC + h * HID:
                                            jl * OC + (h + 1) * HID],
                                    scalar1=p_f[:, 4 * q + h:4 * q + h + 1],
                                    scalar2=None, op0=ap.mult)
                        # scatter into acc
                        for q in range(gs):
                            j = j0 + q
                            nc.tensor.matmul(
                                acc[:], lhsT=ohed_w[:, j * P:(j + 1) * P],
                                rhs=pa[:, q * (OC + 8):q * (OC + 8) + OC + 4],
                                start=(j == 0), stop=(j == cpw - 1))

                    # ---- window finalize ----
                    s_cl = sb.tile([P, 4], f32, name="s_cl")
                    nc.vector.tensor_scalar_max(s_cl[:], acc[:, OC:OC + 4], 1e-30)
                    r_t = sb.tile([P, 4], f32, name="r_t")
                    nc.vector.reciprocal(r_t[:], s_cl[:])
                    o_sb = sb.tile([P, OC], bf16, name="o_sb")
                    nc.vector.tensor_tensor(
                        out=o_sb[:].rearrange("p (h c) -> p h c", h=4),
                        in0=acc[:, 0:OC].rearrange("p (h c) -> p h c", h=4),
                        in1=r_t[:, :, None].to_broadcast([P, 4, HID]),
                        op=ap.mult)
                    if l < 3:
                        sq_t = sb.tile([P, OC], bf16, name="sq_t")
                        nc.scalar.activation(sq_t[:], o_sb[:], act.Square)
                        for cc in range(2):
                            # stats: sum_d o, sum_d o^2 accumulated over windows
                            nc.tensor.matmul(stat_ps[:, cc:cc + 1],
                                             lhsT=o_sb[:, cc * P:(cc + 1) * P],
                                             rhs=ones_col[:, :],
                                             start=False, stop=False)
                            nc.tensor.matmul(stat_ps[:, 2 + cc:3 + cc],
                                             lhsT=sq_t[:, cc * P:(cc + 1) * P],
                                             rhs=ones_col[:, :],
                                             start=False,
                                             stop=(w == W - 1 and cc == 1))
                            # transpose to hpre
                            hp = hpre_a if cc == 0 else hpre_b
                            ps_t = pssm.tile([P, P], f32, name="ps_t", tag="psA")
                            nc.tensor.matmul(ps_t[:], lhsT=o_sb[:, cc * P:(cc + 1) * P],
                                             rhs=ident_bf[:], start=True, stop=True)
                            nc.scalar.copy(hp[:, w * P:(w + 1) * P], ps_t[:])
                    else:
                        u1 = sb.tile([P, HID], f32, name="u1")
                        nc.vector.tensor_tensor(out=u1[:], in0=o_sb[:, 0:HID],
                                                in1=o_sb[:, HID:2 * HID], op=ap.add)
                        u2 = sb.tile([P, HID], f32, name="u2")
                        nc.vector.tensor_tensor(out=u2[:], in0=o_sb[:, 2 * HID:3 * HID],
                                                in1=o_sb[:, 3 * HID:4 * HID], op=ap.add)
                        u3 = sb.tile([P, HID], f32, name="u3")
                        nc.vector.tensor_tensor(out=u3[:], in0=u1[:], in1=u2[:], op=ap.add)
                        nc.vector.tensor_scalar_mul(
                            h3_pre[:, w * HID:(w + 1) * HID], u3[:], 0.25)
                        u3b = sb.tile([P, HID], bf16, name="u3b")
                        nc.vector.tensor_scalar_mul(u3b[:], u3[:], 0.25)
                        sq3 = sb.tile([P, HID], bf16, name="sq3")
                        nc.scalar.activation(sq3[:], u3b[:], act.Square)
                        nc.tensor.matmul(stat_ps[:HID, 0:1], lhsT=u3b[:, :],
                                         rhs=ones_col[:, :],
                                         start=False, stop=False)
                        nc.tensor.matmul(stat_ps[:HID, 2:3], lhsT=sq3[:, :],
                                         rhs=ones_col[:, :],
                                         start=False, stop=False)

                # ---- BN stats allreduce ----
                if l == 3:
                    # close the full-bank group (window matmuls only cover
                    # the first HID partitions)
                    nc.tensor.matmul(stat_ps[:], lhsT=ident_bf[:], rhs=zeros4[:],
                                     start=False, stop=True)
                st_sb = sb.tile([P, 4], f32, name="st_sb")
                nc.vector.tensor_copy(st_sb[:], stat_ps[:])
                if l == 3:
                    nc.vector.memset(st_sb[:, 1:2], 0.0)
                    nc.vector.memset(st_sb[:, 3:4], 0.0)
                nc.sync.dma_start(stats_i[:], st_sb[:])
                nc.gpsimd.collective_compute(
                    "AllReduce", ap.add, replica_groups=rg,
                    ins=[stats_i[:]], outs=[stats_os[l][:]])
                gstats = sb.tile([P, 4], f32, name="gstats")
                nc.sync.dma_start(gstats[:], stats_os[l][:])

                # scale/shift: [P, 2] (cc cols)
                oc_chunks = [j for j, (ll, _) in enumerate(
                    [(0, 0), (0, 1), (1, 0), (1, 1), (2, 0), (2, 1), (3, 0)]) if ll == l]
                mu = sb.tile([P, 2], f32, name="mu")
                nc.vector.tensor_scalar_mul(mu[:], gstats[:, 0:2], 1.0 / N)
                msq = sb.tile([P, 2], f32, name="msq")
                nc.vector.tensor_scalar_mul(msq[:], gstats[:, 2:4], 1.0 / N)
                mu2 = sb.tile([P, 2], f32, name="mu2")
                nc.vector.tensor_tensor(out=mu2[:], in0=mu[:], in1=mu[:], op=ap.mult)
                var = sb.tile([P, 2], f32, name="var")
                nc.vector.tensor_tensor(out=var[:], in0=msq[:], in1=mu2[:], op=ap.subtract)
                vpe = sb.tile([P, 2], f32, name="vpe")
                nc.vector.tensor_scalar_add(vpe[:], var[:], EPS)
                rec = sb.tile([P, 2], f32, name="rec")
                nc.vector.reciprocal(rec[:], vpe[:])
                rstd = sb.tile([P, 2], f32, name="rstd")
                nc.scalar.sqrt(rstd[:], rec[:])
                scal = sb.tile([P, 2], f32, name="scal")
                shif = sb.tile([P, 2], f32, name="shif")
                for i, j in enumerate(oc_chunks):
                    nc.vector.tensor_tensor(out=scal[:, i:i + 1], in0=g_sb[:, j:j + 1],
                                            in1=rstd[:, i:i + 1], op=ap.mult)
                    tmp_ms = sb.tile([P, 1], f32, name="tmp_ms")
                    nc.vector.tensor_tensor(out=tmp_ms[:], in0=mu[:, i:i + 1],
                                            in1=scal[:, i:i + 1], op=ap.mult)
                    nc.vector.tensor_tensor(out=shif[:, i:i + 1], in0=b_sb[:, j:j + 1],
                                            in1=tmp_ms[:], op=ap.subtract)

                # ---- BN apply + elu ----
                if l < 3:
                    for cc, (hp, ht) in enumerate([(hpre_a, hT_a), (hpre_b, hT_b)]):
                        for hh in range(4):
                            hsl = slice(hh * (NPAD // 4), (hh + 1) * (NPAD // 4))
                            t_big = sb.tile([P, NPAD // 4], bf16, name="t_big", bufs=2)
                            nc.vector.tensor_scalar(
                                out=t_big[:], in0=hp[:, hsl], scalar1=scal[:, cc:cc + 1],
                                scalar2=shif[:, cc:cc + 1], op0=ap.mult, op1=ap.add)
                            m_big = sb.tile([P, NPAD // 4], bf16, name="m_big", bufs=2)
                            nc.vector.tensor_scalar_min(m_big[:], t_big[:], 0.0)
                            nc.scalar.activation(m_big[:], m_big[:], act.Exp)
                            nc.vector.tensor_scalar_add(m_big[:], m_big[:], -1.0)
                            nc.vector.tensor_tensor(out=ht[:, hsl], in0=t_big[:], in1=m_big[:], op=ap.max)
                else:
                    # replicate scale/shift rows: [P,1]->[1,P]->K=1 matmul
                    for nm, col in (("scal3", scal), ("shif3", shif)):
                        ps_r = pssm.tile([1, P], f32, name="ps_r", tag="psA")
                        nc.tensor.transpose(ps_r[:], col[:, 0:1], ident_f[:])
                        row_t = sb.tile([1, P], f32, name=nm + "_row")
                        nc.scalar.copy(row_t[:], ps_r[:])
                        ps_rep = pssm.tile([P, HID], f32, name="ps_rep", tag="psA")
                        nc.tensor.matmul(ps_rep[:], lhsT=ones_f[:, :P],
                                         rhs=row_t[:, 0:HID], start=True, stop=True)
                        rep_t = sb.tile([P, HID], f32, name=nm + "_rep", bufs=1)
                        nc.scalar.copy(rep_t[:], ps_rep[:])
                        if nm == "scal3":
                            scal3_rep = rep_t
                        else:
                            shif3_rep = rep_t
                    for w in range(W):
                        sl3 = slice(w * HID, (w + 1) * HID)
                        t3 = sb.tile([P, HID], f32, name="t3")
                        nc.vector.tensor_tensor(out=t3[:], in0=h3_pre[:, sl3],
                                                in1=scal3_rep[:], op=ap.mult)
                        nc.vector.tensor_tensor(out=t3[:], in0=t3[:],
                                                in1=shif3_rep[:], op=ap.add)
                        m3 = sb.tile([P, HID], f32, name="m3")
                        nc.vector.tensor_scalar_min(m3[:], t3[:], 0.0)
                        e3 = sb.tile([P, HID], f32, name="e3")
                        nc.scalar.activation(e3[:], m3[:], act.Exp)
                        nc.vector.tensor_scalar_add(e3[:], e3[:], -1.0)
                        nc.vector.tensor_tensor(out=h3_bn[:, sl3], in0=t3[:],
                                                in1=e3[:], op=ap.max)

            # ---- graph mean pool + MLP (replicated) ----
            ps_pool = psacc.tile([P, HID], f32, name="ps_pool", tag="acc")
            for w in range(W):
                nc.tensor.matmul(ps_pool[:], lhsT=ind_sb[:, w * P:(w + 1) * P],
                                 rhs=h3_bn[:, w * HID:(w + 1) * HID],
                                 start=(w == 0), stop=(w == W - 1))
            pool_sb = sb.tile([P, HID], f32, name="pool_sb")
            nc.vector.tensor_copy(pool_sb[:], ps_pool[:])
            nc.sync.dma_start(pool_i[:], pool_sb[:])
            nc.gpsimd.collective_compute(
                "AllReduce", ap.add, replica_groups=rg,
                ins=[pool_i[:]], outs=[pool_o[:]])
            pool_g = sb.tile([P, HID], f32, name="pool_g")
            nc.sync.dma_start(pool_g[:], pool_o[:])
            pooled = sb.tile([P, HID], f32, name="pooled")
            nc.vector.tensor_scalar_mul(pooled[:], pool_g[:], cnt_sb[:, :1])
            ps_pT = pssm.tile([HID, P], f32, name="ps_pT", tag="psA")
            nc.tensor.transpose(ps_pT[:], pooled[:], ident_f[:])
            pooledT = sb.tile([HID, P], bf16, name="pooledT")
            nc.scalar.copy(pooledT[:], ps_pT[:])
            ps_o1 = pssm.tile([P, HID], f32, name="ps_o1", tag="psA")
            nc.tensor.matmul(ps_o1[:], lhsT=pooledT[:], rhs=fc1w_sb[:, :], start=True, stop=True)
            o1b = sb.tile([P, HID], f32, name="o1b")
            nc.vector.tensor_tensor(out=o1b[:], in0=ps_o1[:], in1=fc1b_sb[:], op=ap.add)
            o1r = sb.tile([P, HID], bf16, name="o1r")
            nc.scalar.activation(o1r[:], o1b[:], act.Relu)
            o1rf = sb.tile([P, HID], f32, name="o1rf")
            nc.vector.tensor_copy(o1rf[:], o1r[:])
            ps_o1T = pssm.tile([HID, P], f32, name="ps_o1T", tag="psA")
            nc.tensor.transpose(ps_o1T[:], o1rf[:], ident_f[:])
            o1T = sb.tile([HID, P], bf16, name="o1T")
            nc.scalar.copy(o1T[:], ps_o1T[:])
            ps_o2 = pssm.tile([P, NCLS], f32, name="ps_o2", tag="psA")
            nc.tensor.matmul(ps_o2[:], lhsT=o1T[:], rhs=fc2w_sb[:, :], start=True, stop=True)
            o2b = sb.tile([P, NCLS], f32, name="o2b")
            nc.vector.tensor_tensor(out=o2b[:], in0=ps_o2[:], in1=fc2b_sb[:], op=ap.add)
            nc.sync.dma_start(out_dram[:, :], o2b[:])

    nc.compile()
    return nc


_PROG_CACHE = {}


def kernel(_trace=False, _tracekw=None, **inputs):
    in_maps, cpw = _host_prep(inputs)
    if cpw not in _PROG_CACHE:
        _PROG_CACHE[cpw] = _build_program(cpw)
    nc = _PROG_CACHE[cpw]
    kw = dict(_tracekw or {})
    res = run_bass_kernel_spmd(nc, in_maps, core_ids=list(range(NCORE)),
                               trace=_trace, **kw)
    out = res.results[0]["out"].astype(np.float32)
    if _trace:
        return out, res
    return out


# revision 22
# speedup vs baseline: 1.6462x; 1.6462x over previous
"""GATv2 4-layer GNN on 8 TRN2 NeuronCores.

Sharding: nodes LPT-packed into 240 balanced (core,window) bins of 128 nodes;
edges (incl. self loops) assigned to the bin of their dst. Per layer: local
node matmuls -> AllGather of bf16 xl -> per-window edge phase:
  - one batched dma_gather of xl[src] rows (A2, [e,c] layout)
  - host-precomputed onehot matrices (both orientations) streamed from DRAM
  - z built TRANSPOSED [c,e] in PSUM: xr-expand matmul + A2-transpose matmuls
  - leaky_relu on ScalarE -> logits via N=4 PE matmuls with att folded into
    a block-diagonal rhs -> exp on ScalarE writes p into pa columns
  - message weighting: 4x 4x-mode tensor_scalar per chunk; scatter via
    onehot matmul into PSUM acc; BN stats via ones-column matmuls.
BN-stat AllReduce; fused BN+elu; graph mean-pool + 2-layer MLP replicated.
"""
import heapq

import numpy as np
import ml_dtypes

import concourse.bass as bass
import concourse.bacc as bacc
import concourse.mybir as mybir
import concourse.tile as tile
from concourse.bass_utils import run_bass_kernel_spmd
from concourse.masks import make_identity

P = 128
NCORE = 8
N = 30000
E = 300000
G = 128
IN = 128
HID = 64
HEADS = 4
NCLS = 10
EPS = 1e-5
SLOPE = 0.2
NPC = N // NCORE            # 3750 real nodes per core
W = 30                      # windows per core
NPAD = W * P                # 3840 padded nodes per core
ROWS = NCORE * NPAD         # 30720 rows in the allgathered xl
OC = HEADS * HID            # 256
# (layer, input-channel-chunk) order for weight / hT chunk layouts
IC_CHUNKS = [(0, 0), (1, 0), (1, 1), (2, 0), (2, 1), (3, 0), (3, 1)]

f32 = mybir.dt.float32
bf16 = mybir.dt.bfloat16
i32 = mybir.dt.int32
i16 = mybir.dt.int16
bf = ml_dtypes.bfloat16


def _pack_nodes(deg):
    """LPT-pack nodes into NCORE*W bins of <=128 nodes, balancing edge load."""
    nbins = NCORE * W
    order = np.argsort(-deg, kind="stable")
    heap = [(0, b) for b in range(nbins)]
    heapq.heapify(heap)
    counts = np.zeros(nbins, np.int64)
    load = np.zeros(nbins, np.int64)
    node_bin = np.empty(N, np.int64)
    node_pos = np.empty(N, np.int64)
    for n in order:
        while True:
            l, b = heapq.heappop(heap)
            if counts[b] < P:
                break
        node_bin[n] = b
        node_pos[n] = counts[b]
        counts[b] += 1
        load[b] += deg[n]
        if counts[b] < P:
            heapq.heappush(heap, (load[b], b))
    return node_bin, node_pos


def _host_prep(inputs):
    x = np.asarray(inputs["x"], np.float32)
    ei = np.asarray(inputs["edge_index"], np.int64)
    batch = np.asarray(inputs["batch"], np.int64)

    src = ei[0]
    dst = ei[1]
    deg = np.bincount(dst, minlength=N) + 1  # +1 self loop
    node_bin, node_pos = _pack_nodes(deg)
    row_of = (node_bin // W) * NPAD + (node_bin % W) * P + node_pos

    # unified edge list: self loops first, then real edges; bin of dst
    e_bin = np.concatenate([node_bin, node_bin[dst]])
    e_srcrow = np.concatenate([row_of, row_of[src]]).astype(np.int64)
    e_dstpos = np.concatenate([node_pos, node_pos[dst]]).astype(np.int64)
    order = np.argsort(e_bin, kind="stable")
    cnt_b = np.bincount(e_bin, minlength=NCORE * W)
    cpw = int(-(-int(cnt_b.max()) // P))     # chunks per window
    NIDX = cpw * P                           # gather idxs per window

    idx_tab = np.zeros((NCORE, P, W * (NIDX // 16)), np.int16)
    ohde_tab = np.zeros((NCORE, P, W * cpw * P), bf)
    ohed_tab = np.zeros((NCORE, P, W * cpw * P), bf)

    bin_starts = np.zeros(NCORE * W + 1, np.int64)
    np.cumsum(cnt_b, out=bin_starts[1:])
    se_srcrow = e_srcrow[order]
    se_dstpos = e_dstpos[order]
    for b in range(NCORE * W):
        c, w = b // W, b % W
        lo, hi = bin_starts[b], bin_starts[b + 1]
        k = int(hi - lo)
        srk = se_srcrow[lo:hi]
        dpk = se_dstpos[lo:hi]
        lin = np.arange(k)
        jj = lin // P
        pp = lin % P
        col = w * (NIDX // 16) + lin // 16
        prow = lin % 16
        for m in range(8):
            idx_tab[c, prow + 16 * m, col] = srk.astype(np.int16)
        ohde_tab[c, dpk, w * cpw * P + lin] = 1.0
        ohed_tab[c, pp, (w * cpw + jj) * P + dpk] = 1.0

    # graph indicator per core: [P pos, W*P] (win w block cols -> graph onehot)
    cores_n = node_bin // W
    wins_n = node_bin % W
    ind_tab = np.zeros((NCORE, P, W * P), bf)
    ind_tab[cores_n, node_pos, wins_n * P + batch] = 1.0

    # x transposed per core, permuted order: [IN, NPAD]
    xT = np.zeros((NCORE, IN, NPAD), bf)
    cols = wins_n * P + node_pos
    xT[cores_n, :, cols] = x.astype(bf)

    # weights (identical on all cores)
    wl_tab = np.zeros((P, len(IC_CHUNKS) * OC), bf)
    wr_tab = np.zeros((P, len(IC_CHUNKS) * OC), bf)
    for j, (l, cc) in enumerate(IC_CHUNKS):
        wl = np.asarray(inputs[f"Wl{l}"], np.float32)
        wr = np.asarray(inputs[f"Wr{l}"], np.float32)
        wl_tab[:, j * OC:(j + 1) * OC] = wl[cc * P:(cc + 1) * P, :].astype(bf)
        wr_tab[:, j * OC:(j + 1) * OC] = wr[cc * P:(cc + 1) * P, :].astype(bf)
    # block-diagonal att rhs: [c_part, 4 layers * 2 cc * 4 heads]
    attbd_tab = np.zeros((P, 4 * 2 * HEADS), bf)
    for l in range(4):
        a = np.asarray(inputs[f"att{l}"], np.float32)  # [HEADS, HID]
        for cc in range(2):
            for cp in range(P):
                ch = cc * P + cp
                h = ch // HID
                attbd_tab[cp, l * 8 + cc * 4 + h] = a[h, ch % HID]
    g_tab = np.ones((P, 7), np.float32)
    b_tab = np.zeros((P, 7), np.float32)
    OCC = [(0, 0), (0, 1), (1, 0), (1, 1), (2, 0), (2, 1), (3, 0)]
    for j, (l, cc) in enumerate(OCC):
        g = np.asarray(inputs[f"bng{l}"], np.float32)
        bb = np.asarray(inputs[f"bnb{l}"], np.float32)
        seg = g[cc * P:(cc + 1) * P]
        g_tab[: len(seg), j] = seg
        segb = bb[cc * P:(cc + 1) * P]
        b_tab[: len(segb), j] = segb

    fc1w = np.asarray(inputs["fc1_w"], np.float32).astype(bf)
    fc2w = np.asarray(inputs["fc2_w"], np.float32).astype(bf)
    fc1b_rep = np.tile(np.asarray(inputs["fc1_b"], np.float32).reshape(1, HID), (P, 1))
    fc2b_rep = np.tile(np.asarray(inputs["fc2_b"], np.float32).reshape(1, NCLS), (P, 1))
    gcnt = np.bincount(batch, minlength=G).astype(np.float32)
    cnt_recip = (1.0 / np.maximum(gcnt, 1.0)).reshape(P, 1).astype(np.float32)

    in_maps = []
    for c in range(NCORE):
        in_maps.append(dict(
            xT=np.ascontiguousarray(xT[c]),
            idx_tab=np.ascontiguousarray(idx_tab[c]),
            ohde_tab=np.ascontiguousarray(ohde_tab[c]),
            ohed_tab=np.ascontiguousarray(ohed_tab[c]),
            ind_tab=np.ascontiguousarray(ind_tab[c]),
            wl_tab=wl_tab, wr_tab=wr_tab, attbd_tab=attbd_tab,
            g_tab=g_tab, b_tab=b_tab,
            fc1w=fc1w, fc2w=fc2w, fc1b_rep=fc1b_rep, fc2b_rep=fc2b_rep,
            cnt_recip=cnt_recip,
        ))
    return in_maps, cpw


def _build_program(cpw):
    NIDX = cpw * P
    nc = bacc.Bacc(num_devices=NCORE, num_swdge_queues=4,
                   dynamic_dma_scratch_size=32768)
    ap = mybir.AluOpType
    act = mybir.ActivationFunctionType
    rg = [list(range(NCORE))]

    # --- external inputs ---
    xT_in = nc.dram_tensor("xT", [IN, NPAD], bf16, kind="ExternalInput")
    idx_in = nc.dram_tensor("idx_tab", [P, W * (NIDX // 16)], i16, kind="ExternalInput")
    ohde_in = nc.dram_tensor("ohde_tab", [P, W * cpw * P], bf16, kind="ExternalInput")
    ohed_in = nc.dram_tensor("ohed_tab", [P, W * cpw * P], bf16, kind="ExternalInput")
    ind_in = nc.dram_tensor("ind_tab", [P, W * P], bf16, kind="ExternalInput")
    wl_in = nc.dram_tensor("wl_tab", [P, len(IC_CHUNKS) * OC], bf16, kind="ExternalInput")
    wr_in = nc.dram_tensor("wr_tab", [P, len(IC_CHUNKS) * OC], bf16, kind="ExternalInput")
    attbd_in = nc.dram_tensor("attbd_tab", [P, 32], bf16, kind="ExternalInput")
    g_in = nc.dram_tensor("g_tab", [P, 7], f32, kind="ExternalInput")
    b_in = nc.dram_tensor("b_tab", [P, 7], f32, kind="ExternalInput")
    fc1w_in = nc.dram_tensor("fc1w", [HID, HID], bf16, kind="ExternalInput")
    fc2w_in = nc.dram_tensor("fc2w", [HID, NCLS], bf16, kind="ExternalInput")
    fc1b_in = nc.dram_tensor("fc1b_rep", [P, HID], f32, kind="ExternalInput")
    fc2b_in = nc.dram_tensor("fc2b_rep", [P, NCLS], f32, kind="ExternalInput")
    cnt_in = nc.dram_tensor("cnt_recip", [P, 1], f32, kind="ExternalInput")
    out_dram = nc.dram_tensor("out", [G, NCLS], f32, kind="ExternalOutput")

    with tile.TileContext(nc) as tc:
        with (
            tc.tile_pool(name="persist", bufs=1) as pers,
            tc.tile_pool(name="sbuf", bufs=3) as sb,
            tc.tile_pool(name="stage", bufs=3) as stg,
            tc.tile_pool(name="psz", bufs=2, space="PSUM") as psz_pool,
            tc.tile_pool(name="psacc", bufs=2, space="PSUM") as psacc,
            tc.tile_pool(name="pssm", bufs=1, space="PSUM") as pssm,
            tc.tile_pool(name="psstat", bufs=1, space="PSUM") as psstat,
            tc.tile_pool(name="dram", bufs=1, space="DRAM") as dr,
        ):
            # ---- persistent tiles / table loads ----
            ident_f = pers.tile([P, P], f32)
            make_identity(nc, ident_f[:])
            ident_bf = pers.tile([P, P], bf16)
            nc.vector.tensor_copy(ident_bf[:], ident_f[:])
            alpha_col = pers.tile([P, 1], f32)
            nc.vector.memset(alpha_col[:], SLOPE)
            ones_f = pers.tile([1, P], f32)
            nc.vector.memset(ones_f[:], 1.0)
            ones_col = pers.tile([P, 1], bf16)
            nc.vector.memset(ones_col[:], 1.0)
            zeros4 = pers.tile([P, 4], bf16)
            nc.vector.memset(zeros4[:], 0.0)

            def load(name, handle, shape, dt):
                t = pers.tile(shape, dt, name=name)
                nc.sync.dma_start(t[:], handle[:, :])
                return t
            xT_sb = load("xT_sb", xT_in, [IN, NPAD], bf16)
            idx_sb = load("idx_sb", idx_in, [P, W * (NIDX // 16)], i16)
            ind_sb = load("ind_sb", ind_in, [P, W * P], bf16)
            wl_sb = load("wl_sb", wl_in, [P, len(IC_CHUNKS) * OC], bf16)
            wr_sb = load("wr_sb", wr_in, [P, len(IC_CHUNKS) * OC], bf16)
            attbd_sb = load("attbd_sb", attbd_in, [P, 32], bf16)
            g_sb = load("g_sb", g_in, [P, 7], f32)
            b_sb = load("b_sb", b_in, [P, 7], f32)
            fc1w_sb = load("fc1w_sb", fc1w_in, [HID, HID], bf16)
            fc2w_sb = load("fc2w_sb", fc2w_in, [HID, NCLS], bf16)
            fc1b_sb = load("fc1b_sb", fc1b_in, [P, HID], f32)
            fc2b_sb = load("fc2b_sb", fc2b_in, [P, NCLS], f32)
            cnt_sb = load("cnt_sb", cnt_in, [P, 1], f32)

            hT_a = pers.tile([P, NPAD], bf16)      # h^T chunk c0:128
            hT_b = pers.tile([P, NPAD], bf16)      # h^T chunk c128:256
            hpre_a = pers.tile([P, NPAD], bf16)    # pre-BN h^T chunk 0
            hpre_b = pers.tile([P, NPAD], bf16)
            h3_pre = pers.tile([P, W * HID], f32)  # layer3 pre-BN, [n, c] layout
            h3_bn = pers.tile([P, W * HID], bf16)
            xr_all = pers.tile([P, W * OC], bf16)
            xl_all = pers.tile([P, W * OC], bf16)

            # DRAM scratch
            xl_loc = dr.tile([NPAD, OC], bf16)
            xl_fulls = [dr.tile([ROWS, OC], bf16, addr_space="Shared", name=f"xl_full{i}")
                        for i in range(4)]
            stats_i = dr.tile([P, 4], f32)
            stats_os = [dr.tile([P, 4], f32, addr_space="Shared", name=f"stats_o{i}")
                        for i in range(4)]
            pool_i = dr.tile([P, HID], f32)
            pool_o = dr.tile([P, HID], f32, addr_space="Shared")

            # gather segments of <=8 chunks (1024-idx dma_gather limit),
            # each split into compute groups of <=4 chunks
            segs = []
            jb = 0
            while jb < cpw:
                sl = min(8, cpw - jb)
                segs.append((jb, sl))
                jb += sl
            groups = []   # (jbase_global, gs, seg_index, jbase_in_seg)
            for si, (jb, sl) in enumerate(segs):
                q = 0
                while q < sl:
                    gs = min(4, sl - q)
                    groups.append((jb + q, gs, si, q))
                    q += gs

            gq = [0]  # global SWDGE gather counter: lane<->queue stays consistent
            for l in range(4):
                ic_chunks = [j for j, (ll, _) in enumerate(IC_CHUNKS) if ll == l]
                hts = [xT_sb] if l == 0 else [hT_a, hT_b]

                # ---- node phase: xl = h @ Wl, xr = h @ Wr ----
                for w in range(W):
                    ps_xl = pssm.tile([P, OC], f32, name="ps_xl", tag="psA")
                    ps_xr = psacc.tile([P, OC], f32, name="ps_xr", tag="acc")
                    for i, j in enumerate(ic_chunks):
                        lhs = hts[i][:, w * P:(w + 1) * P]
                        nc.tensor.matmul(ps_xl[:], lhsT=lhs, rhs=wl_sb[:, j * OC:(j + 1) * OC],
                                         start=(i == 0), stop=(i == len(ic_chunks) - 1))
                        nc.tensor.matmul(ps_xr[:], lhsT=lhs, rhs=wr_sb[:, j * OC:(j + 1) * OC],
                                         start=(i == 0), stop=(i == len(ic_chunks) - 1))
                    xlsl = xl_all[:, w * OC:(w + 1) * OC]
                    nc.scalar.copy(xlsl, ps_xl[:])
                    nc.sync.dma_start(xl_loc[w * P:(w + 1) * P, :], xlsl)
                    nc.vector.tensor_copy(xr_all[:, w * OC:(w + 1) * OC], ps_xr[:])

                # ---- allgather xl ----
                xl_full = xl_fulls[l]
                nc.gpsimd.collective_compute(
                    "AllGather", ap.bypass, replica_groups=rg,
                    ins=[xl_loc[:]], outs=[xl_full[:]],
                )

                # ---- edge phase ----
                attl = attbd_sb[:, l * 8:l * 8 + 8]   # [c, cc*4+h]
                stat_ps = psstat.tile([P, 4], f32, name="stat_ps", tag="stat")
                # open + zero the stat psum bank once; windows accumulate into
                # it with start=False; the last stat matmul closes the group
                nc.tensor.matmul(stat_ps[:], lhsT=ident_bf[:], rhs=zeros4[:],
                                 start=True, stop=False)
                for w in range(W):
                    # batched gathers of xl[src] (<=1024 idxs each)
                    a2tiles = []
                    for si, (jb, sl) in enumerate(segs):
                        A2s = stg.tile([P, sl * OC], bf16, name=f"A2s{si}",
                                       tag=f"A2{si}")
                        nidx_s = sl * P
                        c0 = (w * cpw + jb) * 8
                        nc.gpsimd.dma_gather(
                            A2s[:].rearrange("p (j c) -> p j c", c=OC),
                            xl_full[:, :],
                            idx_sb[:, c0:c0 + nidx_s // 16],
                            nidx_s, nidx_s, OC,
                            queue_num=gq[0] % 4,
                        )
                        gq[0] += 1
                        a2tiles.append(A2s)
                    ohde_w = stg.tile([P, cpw * P], bf16, name="ohde_w", tag="ohde")
                    nc.sync.dma_start(ohde_w[:], ohde_in[:, w * cpw * P:(w + 1) * cpw * P])
                    ohed_w = stg.tile([P, cpw * P], bf16, name="ohed_w", tag="ohed")
                    nc.sync.dma_start(ohed_w[:], ohed_in[:, w * cpw * P:(w + 1) * cpw * P])

                    acc = psacc.tile([P, OC + 4], f32, name="acc", tag="acc")
                    xrw0 = xr_all[:, w * OC:w * OC + P]
                    xrw1 = xr_all[:, w * OC + P:(w + 1) * OC]
                    for (j0, gs, si, qb) in groups:
                        A2t = a2tiles[si]
                        E_g = gs * P
                        zt = psz_pool.tile([P, 2 * E_g], f32, name="zt", tag="zt")
                        lg = pssm.tile([P, 4 * gs], f32, name="lg", tag="psA")
                        # z[c,e] = xr[dst_e,c] + xl[src_e,c], transposed, built
                        # per chunk region: xr-expand opens the psum group,
                        # the A2 transpose-matmul closes it (one open group
                        # per psum bank at a time).
                        for q in range(gs):
                            j = j0 + q
                            jl = qb + q
                            ohq = ohde_w[:, j * P:(j + 1) * P]
                            for cc, xrw in ((0, xrw0), (1, xrw1)):
                                reg = slice(cc * E_g + q * P, cc * E_g + (q + 1) * P)
                                nc.tensor.matmul(zt[:, reg], lhsT=xrw, rhs=ohq,
                                                 start=True, stop=False)
                                nc.tensor.matmul(
                                    zt[:, reg],
                                    lhsT=A2t[:, jl * OC + cc * P:jl * OC + (cc + 1) * P],
                                    rhs=ident_bf[:],
                                    start=False, stop=True)
                        # leaky relu -> bf16
                        zl = sb.tile([P, 2 * E_g], bf16, name="zl", bufs=2)
                        nc.scalar.activation(zl[:, 0:E_g], zt[:, 0:E_g], act.Prelu,
                                             bias=0.0, scale=1.0, alpha=alpha_col[:, :1])
                        nc.scalar.activation(zl[:, E_g:2 * E_g], zt[:, E_g:2 * E_g],
                                             act.Prelu,
                                             bias=0.0, scale=1.0, alpha=alpha_col[:, :1])
                        # logits: lg[e, 4] = sum_c zl[c,e] * att_bd[c,h]
                        for q in range(gs):
                            nc.tensor.matmul(lg[:, 4 * q:4 * q + 4],
                                             lhsT=zl[:, q * P:(q + 1) * P],
                                             rhs=attl[:, 0:4], start=True, stop=False)
                            nc.tensor.matmul(lg[:, 4 * q:4 * q + 4],
                                             lhsT=zl[:, E_g + q * P:E_g + (q + 1) * P],
                                             rhs=attl[:, 4:8], start=False, stop=True)
                        # p = exp(lg) (f32 for tensor_scalar), bf16 copy into pa
                        p_f = sb.tile([P, 4 * gs], f32, name="p_f", bufs=2)
                        nc.scalar.activation(p_f[:], lg[:], act.Exp)
                        pa = sb.tile([P, gs * (OC + 8)], bf16, name="pa", bufs=2)
                        pav = pa[:].rearrange("p (q d) -> p q d", q=gs)
                        nc.vector.tensor_copy(
                            pav[:, :, OC:OC + 4],
                            p_f[:].rearrange("p (q h) -> p q h", q=gs))
                        # weighted messages: one strided tensor_tensor for the
                        # whole group (p broadcast per head via 0-stride view)
                        nc.vector.tensor_tensor(
                            out=pav[:, :, 0:OC].rearrange(
                                "p q (h c) -> p q h c", h=HEADS),
                            in0=A2t[:, qb * OC:(qb + gs) * OC].rearrange(
                                "p (q h c) -> p q h c", q=gs, h=HEADS),
                            in1=p_f[:].rearrange(
                                "p (q h) -> p q h", q=gs)[:, :, :, None]
                                .to_broadcast([P, gs, HEADS, HID]),
                            op=ap.mult)
                        # scatter into acc
                        for q in range(gs):
                            j = j0 + q
                            nc.tensor.matmul(
                                acc[:], lhsT=ohed_w[:, j * P:(j + 1) * P],
                                rhs=pa[:, q * (OC + 8):q * (OC + 8) + OC + 4],
                                start=(j == 0), stop=(j == cpw - 1))

                    # ---- window finalize ----
                    s_cl = sb.tile([P, 4], f32, name="s_cl")
                    nc.vector.tensor_scalar_max(s_cl[:], acc[:, OC:OC + 4], 1e-30)
                    r_t = sb.tile([P, 4], f32, name="r_t")
                    nc.vector.reciprocal(r_t[:], s_cl[:])
                    o_sb = sb.tile([P, OC], bf16, name="o_sb")
                    nc.vector.tensor_tensor(
                        out=o_sb[:].rearrange("p (h c) -> p h c", h=4),
                        in0=acc[:, 0:OC].rearrange("p (h c) -> p h c", h=4),
                        in1=r_t[:, :, None].to_broadcast([P, 4, HID]),
                        op=ap.mult)
                    if l < 3:
                        sq_t = sb.tile([P, OC], bf16, name="sq_t")
                        nc.scalar.activation(sq_t[:], o_sb[:], act.Square)
                        for cc in range(2):
                            # stats: sum_d o, sum_d o^2 accumulated over windows
                            nc.tensor.matmul(stat_ps[:, cc:cc + 1],
                                             lhsT=o_sb[:, cc * P:(cc + 1) * P],
                                             rhs=ones_col[:, :],
                                             start=False, stop=False)
                            nc.tensor.matmul(stat_ps[:, 2 + cc:3 + cc],
                                             lhsT=sq_t[:, cc * P:(cc + 1) * P],
                                             rhs=ones_col[:, :],
                                             start=False,
                                             stop=(w == W - 1 and cc == 1))
                            # transpose to hpre
                            hp = hpre_a if cc == 0 else hpre_b
                            ps_t = pssm.tile([P, P], f32, name="ps_t", tag="psA")
                            nc.tensor.matmul(ps_t[:], lhsT=o_sb[:, cc * P:(cc + 1) * P],
                                             rhs=ident_bf[:], start=True, stop=True)
                            nc.scalar.copy(hp[:, w * P:(w + 1) * P], ps_t[:])
                    else:
                        u1 = sb.tile([P, HID], f32, name="u1")
                        nc.vector.tensor_tensor(out=u1[:], in0=o_sb[:, 0:HID],
                                                in1=o_sb[:, HID:2 * HID], op=ap.add)
                        u2 = sb.tile([P, HID], f32, name="u2")
                        nc.vector.tensor_tensor(out=u2[:], in0=o_sb[:, 2 * HID:3 * HID],
                                                in1=o_sb[:, 3 * HID:4 * HID], op=ap.add)
                        u3 = sb.tile([P, HID], f32, name="u3")
                        nc.vector.tensor_tensor(out=u3[:], in0=u1[:], in1=u2[:], op=ap.add)
                        nc.vector.tensor_scalar_mul(
                            h3_pre[:, w * HID:(w + 1) * HID], u3[:], 0.25)
                        u3b = sb.tile([P, HID], bf16, name="u3b")
                        nc.vector.tensor_scalar_mul(u3b[:], u3[:], 0.25)
                        sq3 = sb.tile([P, HID], bf16, name="sq3")
                        nc.scalar.activation(sq3[:], u3b[:], act.Square)
                        nc.tensor.matmul(stat_ps[:HID, 0:1], lhsT=u3b[:, :],
                                         rhs=ones_col[:, :],
                                         start=False, stop=False)
                        nc.tensor.matmul(stat_ps[:HID, 2:3], lhsT=sq3[:, :],
                                         rhs=ones_col[:, :],
                                         start=False, stop=False)

                # ---- BN stats allreduce ----
                if l == 3:
                    # close the full-bank group (window matmuls only cover
                    # the first HID partitions)
                    nc.tensor.matmul(stat_ps[:], lhsT=ident_bf[:], rhs=zeros4[:],
                                     start=False, stop=True)
                st_sb = sb.tile([P, 4], f32, name="st_sb")
                nc.vector.tensor_copy(st_sb[:], stat_ps[:])
                if l == 3:
                    nc.vector.memset(st_sb[:, 1:2], 0.0)
                    nc.vector.memset(st_sb[:, 3:4], 0.0)
                nc.sync.dma_start(stats_i[:], st_sb[:])
                nc.gpsimd.collective_compute(
                    "AllReduce", ap.add, replica_groups=rg,
                    ins=[stats_i[:]], outs=[stats_os[l][:]])
                gstats = sb.tile([P, 4], f32, name="gstats")
                nc.sync.dma_start(gstats[:], stats_os[l][:])

                # scale/shift: [P, 2] (cc cols)
                oc_chunks = [j for j, (ll, _) in enumerate(
                    [(0, 0), (0, 1), (1, 0), (1, 1), (2, 0), (2, 1), (3, 0)]) if ll == l]
                mu = sb.tile([P, 2], f32, name="mu")
                nc.vector.tensor_scalar_mul(mu[:], gstats[:, 0:2], 1.0 / N)
                msq = sb.tile([P, 2], f32, name="msq")
                nc.vector.tensor_scalar_mul(msq[:], gstats[:, 2:4], 1.0 / N)
                mu2 = sb.tile([P, 2], f32, name="mu2")
                nc.vector.tensor_tensor(out=mu2[:], in0=mu[:], in1=mu[:], op=ap.mult)
                var = sb.tile([P, 2], f32, name="var")
                nc.vector.tensor_tensor(out=var[:], in0=msq[:], in1=mu2[:], op=ap.subtract)
                vpe = sb.tile([P, 2], f32, name="vpe")
                nc.vector.tensor_scalar_add(vpe[:], var[:], EPS)
                rec = sb.tile([P, 2], f32, name="rec")
                nc.vector.reciprocal(rec[:], vpe[:])
                rstd = sb.tile([P, 2], f32, name="rstd")
                nc.scalar.sqrt(rstd[:], rec[:])
                scal = sb.tile([P, 2], f32, name="scal")
                shif = sb.tile([P, 2], f32, name="shif")
                for i, j in enumerate(oc_chunks):
                    nc.vector.tensor_tensor(out=scal[:, i:i + 1], in0=g_sb[:, j:j + 1],
                                            in1=rstd[:, i:i + 1], op=ap.mult)
                    tmp_ms = sb.tile([P, 1], f32, name="tmp_ms")
                    nc.vector.tensor_tensor(out=tmp_ms[:], in0=mu[:, i:i + 1],
                                            in1=scal[:, i:i + 1], op=ap.mult)
                    nc.vector.tensor_tensor(out=shif[:, i:i + 1], in0=b_sb[:, j:j + 1],
                                            in1=tmp_ms[:], op=ap.subtract)

                # ---- BN apply + elu ----
                if l < 3:
                    for cc, (hp, ht) in enumerate([(hpre_a, hT_a), (hpre_b, hT_b)]):
                        for hh in range(4):
                            hsl = slice(hh * (NPAD // 4), (hh + 1) * (NPAD // 4))
                            t_big = sb.tile([P, NPAD // 4], bf16, name="t_big", bufs=2)
                            nc.vector.tensor_scalar(
                                out=t_big[:], in0=hp[:, hsl], scalar1=scal[:, cc:cc + 1],
                                scalar2=shif[:, cc:cc + 1], op0=ap.mult, op1=ap.add)
                            m_big = sb.tile([P, NPAD // 4], bf16, name="m_big", bufs=2)
                            nc.vector.tensor_scalar_min(m_big[:], t_big[:], 0.0)
                            nc.scalar.activation(m_big[:], m_big[:], act.Exp)
                            nc.vector.tensor_scalar_add(m_big[:], m_big[:], -1.0)
                            nc.vector.tensor_tensor(out=ht[:, hsl], in0=t_big[:], in1=m_big[:], op=ap.max)
                else:
                    # replicate scale/shift rows: [P,1]->[1,P]->K=1 matmul
                    for nm, col in (("scal3", scal), ("shif3", shif)):
                        ps_r = pssm.tile([1, P], f32, name="ps_r", tag="psA")
                        nc.tensor.transpose(ps_r[:], col[:, 0:1], ident_f[:])
                        row_t = sb.tile([1, P], f32, name=nm + "_row")
                        nc.scalar.copy(row_t[:], ps_r[:])
                        ps_rep = pssm.tile([P, HID], f32, name="ps_rep", tag="psA")
                        nc.tensor.matmul(ps_rep[:], lhsT=ones_f[:, :P],
                                         rhs=row_t[:, 0:HID], start=True, stop=True)
                        rep_t = sb.tile([P, HID], f32, name=nm + "_rep", bufs=1)
                        nc.scalar.copy(rep_t[:], ps_rep[:])
                        if nm == "scal3":
                            scal3_rep = rep_t
                        else:
                            shif3_rep = rep_t
                    for w in range(W):
                        sl3 = slice(w * HID, (w + 1) * HID)
                        t3 = sb.tile([P, HID], f32, name="t3")
                        nc.vector.tensor_tensor(out=t3[:], in0=h3_pre[:, sl3],
                                                in1=scal3_rep[:], op=ap.mult)
                        nc.vector.tensor_tensor(out=t3[:], in0=t3[:],
                                                in1=shif3_rep[:], op=ap.add)
                        m3 = sb.tile([P, HID], f32, name="m3")
                        nc.vector.tensor_scalar_min(m3[:], t3[:], 0.0)
                        e3 = sb.tile([P, HID], f32, name="e3")
                        nc.scalar.activation(e3[:], m3[:], act.Exp)
                        nc.vector.tensor_scalar_add(e3[:], e3[:], -1.0)
                        nc.vector.tensor_tensor(out=h3_bn[:, sl3], in0=t3[:],
                                                in1=e3[:], op=ap.max)

            # ---- graph mean pool + MLP (replicated) ----
            ps_pool = psacc.tile([P, HID], f32, name="ps_pool", tag="acc")
            for w in range(W):
                nc.tensor.matmul(ps_pool[:], lhsT=ind_sb[:, w * P:(w + 1) * P],
                                 rhs=h3_bn[:, w * HID:(w + 1) * HID],
                                 start=(w == 0), stop=(w == W - 1))
            pool_sb = sb.tile([P, HID], f32, name="pool_sb")
            nc.vector.tensor_copy(pool_sb[:], ps_pool[:])
            nc.sync.dma_start(pool_i[:], pool_sb[:])
            nc.gpsimd.collective_compute(
                "AllReduce", ap.add, replica_groups=rg,
                ins=[pool_i[:]], outs=[pool_o[:]])
            pool_g = sb.tile([P, HID], f32, name="pool_g")
            nc.sync.dma_start(pool_g[:], pool_o[:])
            pooled = sb.tile([P, HID], f32, name="pooled")
            nc.vector.tensor_scalar_mul(pooled[:], pool_g[:], cnt_sb[:, :1])
            ps_pT = pssm.tile([HID, P], f32, name="ps_pT", tag="psA")
            nc.tensor.transpose(ps_pT[:], pooled[:], ident_f[:])
            pooledT = sb.tile([HID, P], bf16, name="pooledT")
            nc.scalar.copy(pooledT[:], ps_pT[:])
            ps_o1 = pssm.tile([P, HID], f32, name="ps_o1", tag="psA")
            nc.tensor.matmul(ps_o1[:], lhsT=pooledT[:], rhs=fc1w_sb[:, :], start=True, stop=True)
            o1b = sb.tile([P, HID], f32, name="o1b")
            nc.vector.tensor_tensor(out=o1b[:], in0=ps_o1[:], in1=fc1b_sb[:], op=ap.add)
            o1r = sb.tile([P, HID], bf16, name="o1r")
            nc.scalar.activation(o1r[:], o1b[:], act.Relu)
            o1rf = sb.tile([P, HID], f32, name="o1rf")
            nc.vector.tensor_copy(o1rf[:], o1r[:])
            ps_o1T = pssm.tile([HID, P], f32, name="ps_o1T", tag="psA")
            nc.tensor.transpose(ps_o1T[:], o1rf[:], ident_f[:])
            o1T = sb.tile([HID, P], bf16, name="o1T")
            nc.scalar.copy(o1T[:], ps_o1T[:])
            ps_o2 = pssm.tile([P, NCLS], f32, name="ps_o2", tag="psA")
            nc.tensor.matmul(ps_o2[:], lhsT=o1T[:], rhs=fc2w_sb[:, :], start=True, stop=True)
            o2b = sb.tile([P, NCLS], f32, name="o2b")
            nc.vector.tensor_tensor(out=o2b[:], in0=ps_o2[:], in1=fc2b_sb[:], op=ap.add)
            nc.sync.dma_start(out_dram[:, :], o2b[:])

    nc.compile()
    return nc


_PROG_CACHE = {}


def kernel(_trace=False, _tracekw=None, **inputs):
    in_maps, cpw = _host_prep(inputs)
    if cpw not in _PROG_CACHE:
        _PROG_CACHE[cpw] = _build_program(cpw)
    nc = _PROG_CACHE[cpw]
    kw = dict(_tracekw or {})
    res = run_bass_kernel_spmd(nc, in_maps, core_ids=list(range(NCORE)),
                               trace=_trace, **kw)
    out = res.results[0]["out"].astype(np.float32)
    if _trace:
        return out, res
    return out
